# revision 2
# baseline (speedup 1.0000x reference)
"""CAM (channel attention) kernel for Trainium2, SPMD over 8 NeuronCores.

Problem: x [16, 512, 64, 64] fp32, gamma [1] fp32.
  q = x.reshape(B, C, N);  energy = q @ q^T          (C x C, contract over N=4096)
  attention = softmax(max(energy, -1, keepdims) - energy, -1)
  out = attention @ q;  result = gamma * out + x

Sharding: data-parallel over batch. 16 batches / 8 cores = 2 batches per core.
gamma replicated. Each core computes its own C x C attention per batch.

Math used here: energy is symmetric, and
  softmax(m[c] - energy[c, :]) = exp(mn[c] - energy[c, :]) / sum(...)
with mn[c] = min_d energy[c, d]  (the jax softmax's internal max-shift turns the
row-max of (m - e) into the row-min of e). All exp args are <= 0 -> no overflow.

Pipeline per batch (per core):
  1. DMA x fp32 -> SBUF (16 tiles [128, 1024]); cast to bf16 (DVE) -> q_bf.
  2. xbar DMA-transpose q_bf -> qT [128, ci, k, 128] bf16 (contract dim on
     partitions; the n <-> (k, p) bijection is consistent between lhsT and rhs
     so the contraction is correct regardless of the xbar's internal order).
  3. mm1: energy[mi] (PSUM fp32, 4 banks) += qT[:, mi, k, :]^T @ qT[:, :, k, :].
  4. softmax: row-min (DVE) -> exp(mn - e) bf16 + row-sum S (ScalarE, one op).
  5. xbar DMA-transpose A -> AT (16x [128, 128] 2D transposes, unambiguous).
  6. mm2: out_tile (PSUM) += AT[mi][dj]^T @ q_bf[dj][:, nt].
  7. epilogue: t = psum * (gamma/S) (ScalarE), out = t + x (DVE), DMA out.
"""

import sys

if "/opt/trn_rl_repo" not in sys.path:
    sys.path.insert(0, "/opt/trn_rl_repo")

import numpy as np

import concourse.bacc as bacc
import concourse.mybir as mybir
import concourse.tile as tile
from concourse.bass_utils import run_bass_kernel_spmd

# Problem constants (hardcoded; kernel.py must be self-contained).
B, C, H, W = 16, 512, 64, 64
N = H * W                      # 4096
N_CORES = 8
BPC = B // N_CORES             # batches per core = 2
CB = C // 128                  # c-blocks = 4
NK = N // 128                  # contraction chunks for mm1 = 32
NS = 4                         # x load slabs per c-block (1024 wide each)
NT = N // 512                  # mm2 output tiles per c-block = 8

F32 = mybir.dt.float32
BF16 = mybir.dt.bfloat16

_PROGRAM = None


def _build_program():
    nc = bacc.Bacc("TRN2", target_bir_lowering=False, debug=True)
    x = nc.declare_dram_parameter("x", [BPC, C, N], F32, isOutput=False)
    gamma = nc.declare_dram_parameter("gamma", [1], F32, isOutput=False)
    out = nc.declare_dram_parameter("out", [BPC, C, N], F32, isOutput=True)

    with tile.TileContext(nc) as tc:
        with (
            tc.tile_pool(name="xf", bufs=22) as xf_pool,
            tc.tile_pool(name="qbf", bufs=5) as qbf_pool,
            tc.tile_pool(name="qt", bufs=1) as qt_pool,
            tc.tile_pool(name="attn", bufs=4) as a_pool,
            tc.tile_pool(name="att", bufs=20) as at_pool,
            tc.tile_pool(name="stat", bufs=24) as stat_pool,
            tc.tile_pool(name="osb", bufs=8) as osb_pool,
            tc.tile_pool(name="const", bufs=2) as const_pool,
            tc.tile_pool(name="ps1", bufs=4, space="PSUM") as ps1_pool,
            tc.tile_pool(name="ps2", bufs=4, space="PSUM") as ps2_pool,
        ):
            # gamma -> all 128 partitions
            gsb = const_pool.tile([1, 1], F32, tag="gsb", name="gsb")
            nc.sync.dma_start(gsb[:, :], gamma[None, :])
            gb = const_pool.tile([128, 1], F32, tag="gb", name="gb")
            nc.gpsimd.partition_broadcast(gb[:, :], gsb[:, :])

            for b in range(BPC):
                # ---- load + cast + transpose ----
                x_tiles = {}
                qbf = [qbf_pool.tile([128, N], BF16, tag="qbf", name="qbf") for _ in range(CB)]
                qt = qt_pool.tile([128, CB, NK, 128], BF16, tag="qt", name="qt")
                for ns in range(NS):
                    for ci in range(CB):
                        xt = xf_pool.tile([128, 1024], F32, tag="xf", name="xf")
                        nc.sync.dma_start(
                            xt[:, :],
                            x[b, ci * 128 : (ci + 1) * 128, ns * 1024 : (ns + 1) * 1024],
                        )
                        x_tiles[ci, ns] = xt
                        nc.vector.tensor_copy(
                            qbf[ci][:, ns * 1024 : (ns + 1) * 1024], xt[:, :]
                        )
                        nc.sync.dma_start_transpose(
                            qt[:, ci, ns * 8 : (ns + 1) * 8, :],
                            qbf[ci][:, ns * 1024 : (ns + 1) * 1024],
                        )

                # ---- mm1: energy (k-outer for early start; staggered tail) ----
                ps1 = [ps1_pool.tile([128, 512], F32, tag="ps1", name="ps1") for _ in range(CB)]
                K_TAIL = 4
                for k in range(NK - K_TAIL):
                    for mi in range(CB):
                        nc.tensor.matmul(
                            ps1[mi][:, :],
                            qt[:, mi, k, :],
                            qt[:, :, k, :],
                            start=(k == 0),
                            stop=False,
                        )

                # ---- softmax per mi, emitted right after that mi's tail ----
                rg = []       # gamma / S per c-block, [128, 1] f32
                at = []       # AT[mi][dj]: [128, 128] bf16
                for mi in range(CB):
                    for k in range(NK - K_TAIL, NK):
                        nc.tensor.matmul(
                            ps1[mi][:, :],
                            qt[:, mi, k, :],
                            qt[:, :, k, :],
                            start=False,
                            stop=(k == NK - 1),
                        )
                    mn = stat_pool.tile([128, 1], F32, tag="mn", name="mn")
                    nc.vector.tensor_reduce(
                        mn[:, :], ps1[mi][:, :],
                        axis=mybir.AxisListType.X, op=mybir.AluOpType.min,
                    )
                    a_t = a_pool.tile([128, 512], BF16, tag="attn", name="attn")
                    s_t = stat_pool.tile([128, 1], F32, tag="s", name="s")
                    nc.scalar.activation(
                        a_t[:, :], ps1[mi][:, :],
                        mybir.ActivationFunctionType.Exp,
                        bias=mn[:, :], scale=-1.0, accum_out=s_t[:, :],
                    )
                    rs = stat_pool.tile([128, 1], F32, tag="rs", name="rs")
                    nc.vector.reciprocal(rs[:, :], s_t[:, :])
                    rg_t = stat_pool.tile([128, 1], F32, tag="rg", name="rg")
                    nc.vector.tensor_tensor(
                        rg_t[:, :], rs[:, :], gb[:, :], op=mybir.AluOpType.mult
                    )
                    rg.append(rg_t)
                    row = []
                    for dj in range(CB):
                        t = at_pool.tile([128, 128], BF16, tag="att", name="att")
                        nc.sync.dma_start_transpose(
                            t[:, :], a_t[:, dj * 128 : (dj + 1) * 128]
                        )
                        row.append(t)
                    at.append(row)

                # ---- mm2 + epilogue ----
                for mi in range(CB):
                    for nt in range(NT):
                        ps2 = ps2_pool.tile([128, 512], F32, tag="ps2", name="ps2")
                        for dj in range(CB):
                            nc.tensor.matmul(
                                ps2[:, :],
                                at[mi][dj][:, :],
                                qbf[dj][:, nt * 512 : (nt + 1) * 512],
                                start=(dj == 0),
                                stop=(dj == CB - 1),
                            )
                        t_sb = osb_pool.tile([128, 512], F32, tag="tsb", name="tsb")
                        nc.scalar.activation(
                            t_sb[:, :], ps2[:, :],
                            mybir.ActivationFunctionType.Copy,
                            bias=0.0, scale=rg[mi][:, :],
                        )
                        o_sb = osb_pool.tile([128, 512], F32, tag="osb", name="osb")
                        xsl = x_tiles[mi, nt // 2][
                            :, (nt % 2) * 512 : (nt % 2 + 1) * 512
                        ]
                        nc.vector.tensor_tensor(
                            o_sb[:, :], t_sb[:, :], xsl, op=mybir.AluOpType.add
                        )
                        nc.sync.dma_start(
                            out[b, mi * 128 : (mi + 1) * 128, nt * 512 : (nt + 1) * 512],
                            o_sb[:, :],
                        )

    nc.finalize()
    return nc


def _get_program():
    global _PROGRAM
    if _PROGRAM is None:
        _PROGRAM = _build_program()
    return _PROGRAM


def _run(x, gamma, trace=False, tmpdir=None):
    """x: [B, C, H, W] fp32, gamma: [1] fp32 -> ([B, C, H, W] fp32, exec_time_ns)"""
    x = np.ascontiguousarray(np.asarray(x, dtype=np.float32)).reshape(B, C, N)
    gamma = np.ascontiguousarray(np.asarray(gamma, dtype=np.float32)).reshape(1)
    nc = _get_program()
    in_maps = [
        {"x": x[i * BPC : (i + 1) * BPC], "gamma": gamma} for i in range(N_CORES)
    ]
    res = run_bass_kernel_spmd(
        nc, in_maps, list(range(N_CORES)), trace=trace, tmpdir=tmpdir
    )
    full = np.concatenate([res.results[i]["out"] for i in range(N_CORES)], axis=0)
    return full.reshape(B, C, H, W), res.exec_time_ns


def kernel(**inputs):
    out, _ = _run(inputs["x"], inputs["gamma"])
    return out


if __name__ == "__main__":
    rng = np.random.default_rng(0)
    x = rng.standard_normal((B, C, H, W), dtype=np.float32)
    gamma = np.zeros((1,), dtype=np.float32)
    out, t = _run(x, gamma)
    print("exec_time_ns:", t)
    print("max |out - x| (gamma=0):", np.abs(out - x).max())


# revision 3
# speedup vs baseline: 1.0767x; 1.0767x over previous
"""CAM (channel attention) kernel for Trainium2, SPMD over 8 NeuronCores.

Problem: x [16, 512, 64, 64] fp32, gamma [1] fp32.
  q = x.reshape(B, C, N);  energy = q @ q^T          (C x C, contract over N=4096)
  attention = softmax(max(energy, -1, keepdims) - energy, -1)
  out = attention @ q;  result = gamma * out + x

Sharding: data-parallel over batch. 16 batches / 8 cores = 2 batches per core.
gamma replicated. Each core computes its own C x C attention per batch.

Math: energy is symmetric, and
  softmax(m[c] - energy[c, :]) = exp(mn[c] - energy[c, :]) / sum(...)
with mn[c] = min_d energy[c, d] (jax softmax's internal max-shift turns the
row-max of (m - e) into the row-min of e). All exp args <= 0 -> no overflow.

Engine layout (chosen to keep every instruction stream free of long waits —
HWDGE rings and engine queues execute in order, so a waiting instruction
blocks everything behind it on that engine):
  SP ring   : x loads, qT xbar-transposes (bf16, [128, 2048] each)
  ACT ring  : out stores; ACT compute: exp(+row-sum), half the psum scaling
  GpSimd    : fp32->bf16 casts (keeps DVE free; GpSimd is otherwise idle)
  DVE       : row-min, AT psum->sbuf copies, epilogue (psum*rg)+x
  PE        : mm1, 16x 128x128 transpose-mode ops for attention^T, mm2
"""

import sys

if "/opt/trn_rl_repo" not in sys.path:
    sys.path.insert(0, "/opt/trn_rl_repo")

import numpy as np

import concourse.bacc as bacc
import concourse.mybir as mybir
import concourse.tile as tile
from concourse.bass_utils import run_bass_kernel_spmd
from concourse.masks import make_identity

# Problem constants (hardcoded; kernel.py must be self-contained).
B, C, H, W = 16, 512, 64, 64
N = H * W                      # 4096
N_CORES = 8
BPC = B // N_CORES             # batches per core = 2
CB = C // 128                  # c-blocks = 4
NK = N // 128                  # contraction chunks for mm1 = 32
NT = N // 512                  # mm2 output tiles per c-block = 8

F32 = mybir.dt.float32
BF16 = mybir.dt.bfloat16

_PROGRAM = None


def _build_program():
    nc = bacc.Bacc("TRN2", target_bir_lowering=False, debug=True)
    x = nc.declare_dram_parameter("x", [BPC, C, N], F32, isOutput=False)
    gamma = nc.declare_dram_parameter("gamma", [1], F32, isOutput=False)
    out = nc.declare_dram_parameter("out", [BPC, C, N], F32, isOutput=True)

    with tile.TileContext(nc) as tc:
        with (
            tc.tile_pool(name="xf", bufs=20) as xf_pool,
            tc.tile_pool(name="qbf", bufs=10) as qbf_pool,
            tc.tile_pool(name="qt0", bufs=1) as qt0_pool,
            tc.tile_pool(name="qt1", bufs=1) as qt1_pool,
            tc.tile_pool(name="attn", bufs=4) as a_pool,
            tc.tile_pool(name="att", bufs=20) as at_pool,
            tc.tile_pool(name="stat", bufs=24) as stat_pool,
            tc.tile_pool(name="stage", bufs=4) as stage_pool,
            tc.tile_pool(name="tsb", bufs=4) as tsb_pool,
            tc.tile_pool(name="const", bufs=1) as const_pool,
            tc.tile_pool(name="ps1", bufs=4, space="PSUM") as ps1_pool,
            tc.tile_pool(name="ps2", bufs=4, space="PSUM") as ps2_pool,
        ):
            # constants: gamma broadcast + identity for PE transposes
            gsb = const_pool.tile([1, 1], F32, tag="gsb", name="gsb")
            nc.sync.dma_start(gsb[:, :], gamma[None, :])
            gb = const_pool.tile([128, 1], F32, tag="gb", name="gb")
            nc.gpsimd.partition_broadcast(gb[:, :], gsb[:, :])
            ident = const_pool.tile([128, 128], BF16, tag="ident", name="ident")
            make_identity(nc, ident[:, :])

            for b in range(BPC):
                # ---- prep: load x, cast bf16, xbar-transpose to qT ----
                # h-outer so all 4 c-blocks of the first half transpose first
                # and mm1 can start at ~1/2 of the x load.
                x_tiles = {}
                qbf = {}   # (ci, h) -> [128, 2048] bf16
                qt = []    # h -> [128, CB, 16, 128] bf16
                for h in range(2):
                    pool = qt0_pool if h == 0 else qt1_pool
                    qt.append(
                        pool.tile([128, CB, NK // 2, 128], BF16,
                                  tag=f"qt{h}", name=f"qt{h}")
                    )
                for h in range(2):
                    for ci in range(CB):
                        qb = qbf_pool.tile([128, 2048], BF16, tag="qbf", name="qbf")
                        qbf[ci, h] = qb
                        for s in range(2):
                            ns = h * 2 + s
                            xt = xf_pool.tile([128, 1024], F32, tag="xf", name="xf")
                            nc.sync.dma_start(
                                xt[:, :],
                                x[b, ci * 128 : (ci + 1) * 128,
                                  ns * 1024 : (ns + 1) * 1024],
                            )
                            x_tiles[ci, ns] = xt
                            nc.gpsimd.tensor_copy(
                                qb[:, s * 1024 : (s + 1) * 1024], xt[:, :]
                            )
                        nc.sync.dma_start_transpose(qt[h][:, ci, :, :], qb[:, :])

                # ---- mm1: energy, 4 PSUM banks, k-outer; per-mi staggered tail ----
                ps1 = [
                    ps1_pool.tile([128, 512], F32, tag="ps1", name="ps1")
                    for _ in range(CB)
                ]
                K_TAIL = 4  # last chunks done mi-contiguous so mi=0 finishes first
                for k in range(NK - K_TAIL):
                    h, kk = divmod(k, NK // 2)
                    for mi in range(CB):
                        nc.tensor.matmul(
                            ps1[mi][:, :],
                            qt[h][:, mi, kk, :],
                            qt[h][:, :, kk, :],
                            start=(k == 0),
                            stop=False,
                        )
                rgs = []
                a_ts = []
                for mi in range(CB):
                    for k in range(NK - K_TAIL, NK):
                        h, kk = divmod(k, NK // 2)
                        nc.tensor.matmul(
                            ps1[mi][:, :],
                            qt[h][:, mi, kk, :],
                            qt[h][:, :, kk, :],
                            start=False,
                            stop=(k == NK - 1),
                        )
                    # softmax stats for this block (DVE + ACT run during the
                    # remaining tails)
                    mn = stat_pool.tile([128, 1], F32, tag="mn", name="mn")
                    nc.vector.tensor_reduce(
                        mn[:, :], ps1[mi][:, :],
                        axis=mybir.AxisListType.X, op=mybir.AluOpType.min,
                    )
                    a_t = a_pool.tile([128, 512], BF16, tag="attn", name="attn")
                    s_t = stat_pool.tile([128, 1], F32, tag="s", name="s")
                    nc.scalar.activation(
                        a_t[:, :], ps1[mi][:, :],
                        mybir.ActivationFunctionType.Exp,
                        bias=mn[:, :], scale=-1.0, accum_out=s_t[:, :],
                    )
                    a_ts.append(a_t)
                    rs = stat_pool.tile([128, 1], F32, tag="rs", name="rs")
                    nc.vector.reciprocal(rs[:, :], s_t[:, :])
                    rg_t = stat_pool.tile([128, 1], F32, tag="rg", name="rg")
                    nc.vector.tensor_tensor(
                        rg_t[:, :], rs[:, :], gb[:, :], op=mybir.AluOpType.mult
                    )
                    rgs.append(rg_t)

                # ---- attention^T via PE transpose-mode (PSUM slots from ps2) ----
                at = [[None] * CB for _ in range(CB)]
                for mi in range(CB):
                    for dj in range(CB):
                        pst = ps2_pool.tile([128, 128], BF16, tag="ps2", name="atp")
                        nc.tensor.transpose(
                            pst[:, :],
                            a_ts[mi][:, dj * 128 : (dj + 1) * 128],
                            ident[:, :],
                        )
                        t_sb = at_pool.tile([128, 128], BF16, tag="att", name="att")
                        nc.vector.tensor_copy(t_sb[:, :], pst[:, :])
                        at[mi][dj] = t_sb

                # ---- mm2 + epilogue (nt-outer: frees qbf/x low half early) ----
                stage = {}
                for nt in range(NT):
                    hh = nt // 4
                    off = (nt % 4) * 512
                    for mi in range(CB):
                        if nt % 2 == 0:
                            stage[mi] = stage_pool.tile(
                                [128, 1024], F32, tag="stage", name="stage"
                            )
                        ps2 = ps2_pool.tile([128, 512], F32, tag="ps2", name="ps2")
                        for dj in range(CB):
                            nc.tensor.matmul(
                                ps2[:, :],
                                at[mi][dj][:, :],
                                qbf[dj, hh][:, off : off + 512],
                                start=(dj == 0),
                                stop=(dj == CB - 1),
                            )
                        xsl = x_tiles[mi, nt // 2][:, (nt % 2) * 512 : (nt % 2 + 1) * 512]
                        dst = stage[mi][:, (nt % 2) * 512 : (nt % 2 + 1) * 512]
                        if (nt + mi) % 2 == 0:
                            # one fused DVE op: (psum * rg) + x
                            nc.vector.scalar_tensor_tensor(
                                dst, ps2[:, :], rgs[mi][:, :], xsl,
                                op0=mybir.AluOpType.mult, op1=mybir.AluOpType.add,
                            )
                        else:
                            # split across ACT (scale) + DVE (add)
                            t_sb = tsb_pool.tile([128, 512], F32, tag="tsb", name="tsb")
                            nc.scalar.activation(
                                t_sb[:, :], ps2[:, :],
                                mybir.ActivationFunctionType.Copy,
                                bias=0.0, scale=rgs[mi][:, :],
                            )
                            nc.vector.tensor_tensor(
                                dst, t_sb[:, :], xsl, op=mybir.AluOpType.add
                            )
                        if nt % 2 == 1:
                            nc.scalar.dma_start(
                                out[b, mi * 128 : (mi + 1) * 128,
                                    (nt - 1) * 512 : (nt + 1) * 512],
                                stage[mi][:, :],
                            )

    nc.finalize()
    return nc


def _get_program():
    global _PROGRAM
    if _PROGRAM is None:
        _PROGRAM = _build_program()
    return _PROGRAM


def _run(x, gamma, trace=False, tmpdir=None):
    """x: [B, C, H, W] fp32, gamma: [1] fp32 -> ([B, C, H, W] fp32, exec_time_ns)"""
    x = np.ascontiguousarray(np.asarray(x, dtype=np.float32)).reshape(B, C, N)
    gamma = np.ascontiguousarray(np.asarray(gamma, dtype=np.float32)).reshape(1)
    nc = _get_program()
    in_maps = [
        {"x": x[i * BPC : (i + 1) * BPC], "gamma": gamma} for i in range(N_CORES)
    ]
    res = run_bass_kernel_spmd(
        nc, in_maps, list(range(N_CORES)), trace=trace, tmpdir=tmpdir
    )
    full = np.concatenate([res.results[i]["out"] for i in range(N_CORES)], axis=0)
    return full.reshape(B, C, H, W), res.exec_time_ns


def kernel(**inputs):
    out, _ = _run(inputs["x"], inputs["gamma"])
    return out


if __name__ == "__main__":
    rng = np.random.default_rng(0)
    x = rng.standard_normal((B, C, H, W), dtype=np.float32)
    gamma = np.zeros((1,), dtype=np.float32)
    out, t = _run(x, gamma)
    print("exec_time_ns:", t)
    print("max |out - x| (gamma=0):", np.abs(out - x).max())


# revision 4
# speedup vs baseline: 1.2835x; 1.1920x over previous
"""CAM (channel attention) kernel for Trainium2, SPMD over 8 NeuronCores.

Problem: x [16, 512, 64, 64] fp32, gamma [1] fp32.
  q = x.reshape(B, C, N);  energy = q @ q^T          (C x C, contract over N=4096)
  attention = softmax(max(energy, -1, keepdims) - energy, -1)
  out = attention @ q;  result = gamma * out + x

Sharding: data-parallel over batch. 16 batches / 8 cores = 2 batches per core.
gamma replicated. Each core computes its own C x C attention per batch.

Math: energy is symmetric, and
  softmax(m[c] - energy[c, :]) = exp(mn[c] - energy[c, :]) / sum(...)
with mn[c] = min_d energy[c, d] (jax softmax's internal max-shift turns the
row-max of (m - e) into the row-min of e). All exp args <= 0 -> no overflow.

Engine layout (engine queues and HWDGE rings execute in order, so a waiting
instruction blocks everything behind it on the same engine — streams are
arranged so no long wait sits in front of independent work):
  SP ring   : x loads only (no waits -> back-to-back dispatch)
  ACT ring  : qT xbar-transposes, out stores; ACT compute: exp(+row-sum)
  DVE       : fp32->bf16 casts, row-min, AT psum->sbuf copies,
              epilogue (psum*rg)+x as one fused op
  PE        : mm1, 16x 128x128 transpose-mode ops for attention^T, mm2

Cross-batch pipelining is done by *emission order* (Tile priority == program
order): batch b+1's load/cast/transpose for the low half of qT is emitted in
the middle of batch b's mm2 loop, right where the SBUF slots it needs get
released (qbf/x low halves die after the nt=3 round of the nt-outer mm2).
"""

import sys

if "/opt/trn_rl_repo" not in sys.path:
    sys.path.insert(0, "/opt/trn_rl_repo")

import numpy as np

import concourse.bacc as bacc
import concourse.mybir as mybir
import concourse.tile as tile
from concourse.bass_utils import run_bass_kernel_spmd
from concourse.masks import make_identity

# Problem constants (hardcoded; kernel.py must be self-contained).
B, C, H, W = 16, 512, 64, 64
N = H * W                      # 4096
N_CORES = 8
BPC = B // N_CORES             # batches per core = 2
CB = C // 128                  # c-blocks = 4
NK = N // 128                  # contraction chunks for mm1 = 32
NT = N // 512                  # mm2 output tiles per c-block = 8

F32 = mybir.dt.float32
BF16 = mybir.dt.bfloat16

_PROGRAM = None


def _build_program():
    nc = bacc.Bacc("TRN2", target_bir_lowering=False, debug=True)
    x = nc.declare_dram_parameter("x", [BPC, C, N], F32, isOutput=False)
    gamma = nc.declare_dram_parameter("gamma", [1], F32, isOutput=False)
    out = nc.declare_dram_parameter("out", [BPC, C, N], F32, isOutput=True)

    with tile.TileContext(nc) as tc:
        with (
            tc.tile_pool(name="xf", bufs=20) as xf_pool,
            tc.tile_pool(name="qbf", bufs=12) as qbf_pool,
            tc.tile_pool(name="qt0", bufs=1) as qt0_pool,
            tc.tile_pool(name="qt1", bufs=1) as qt1_pool,
            tc.tile_pool(name="attn", bufs=4) as a_pool,
            tc.tile_pool(name="att", bufs=20) as at_pool,
            tc.tile_pool(name="stat", bufs=24) as stat_pool,
            tc.tile_pool(name="stage", bufs=4) as stage_pool,
            tc.tile_pool(name="const", bufs=1) as const_pool,
            tc.tile_pool(name="ps1", bufs=4, space="PSUM") as ps1_pool,
            tc.tile_pool(name="ps2", bufs=4, space="PSUM") as ps2_pool,
        ):
            # constants: gamma broadcast + identity for PE transposes
            gsb = const_pool.tile([1, 1], F32, tag="gsb", name="gsb")
            nc.sync.dma_start(gsb[:, :], gamma[None, :])
            gb = const_pool.tile([128, 1], F32, tag="gb", name="gb")
            nc.gpsimd.partition_broadcast(gb[:, :], gsb[:, :])
            ident = const_pool.tile([128, 128], BF16, tag="ident", name="ident")
            make_identity(nc, ident[:, :])

            # per-batch state
            x_tiles = [{} for _ in range(BPC)]
            qbf = [{} for _ in range(BPC)]
            qt = [[None, None] for _ in range(BPC)]

            def prep_half(b, h):
                """Load x[b] half h, cast to bf16, xbar-transpose into qt[b][h]."""
                pool = qt0_pool if h == 0 else qt1_pool
                qt[b][h] = pool.tile(
                    [128, CB, NK // 2, 128], BF16, tag=f"qt{h}", name=f"qt{h}"
                )
                for ci in range(CB):
                    qb = qbf_pool.tile([128, 2048], BF16, tag="qbf", name="qbf")
                    qbf[b][ci, h] = qb
                    for s in range(2):
                        ns = h * 2 + s
                        xt = xf_pool.tile([128, 1024], F32, tag="xf", name="xf")
                        nc.sync.dma_start(
                            xt[:, :],
                            x[b, ci * 128 : (ci + 1) * 128,
                              ns * 1024 : (ns + 1) * 1024],
                        )
                        x_tiles[b][ci, ns] = xt
                        nc.vector.tensor_copy(
                            qb[:, s * 1024 : (s + 1) * 1024], xt[:, :]
                        )
                    nc.scalar.dma_start_transpose(qt[b][h][:, ci, :, :], qb[:, :])

            def mm1_softmax(b):
                """energy -> softmax stats -> attention^T tiles (PE transpose)."""
                ps1 = [
                    ps1_pool.tile([128, 512], F32, tag="ps1", name="ps1")
                    for _ in range(CB)
                ]
                K_TAIL = 4
                for k in range(NK - K_TAIL):
                    h, kk = divmod(k, NK // 2)
                    for mi in range(CB):
                        nc.tensor.matmul(
                            ps1[mi][:, :],
                            qt[b][h][:, mi, kk, :],
                            qt[b][h][:, :, kk, :],
                            start=(k == 0),
                            stop=False,
                        )
                rgs, a_ts = [], []
                for mi in range(CB):
                    for k in range(NK - K_TAIL, NK):
                        h, kk = divmod(k, NK // 2)
                        nc.tensor.matmul(
                            ps1[mi][:, :],
                            qt[b][h][:, mi, kk, :],
                            qt[b][h][:, :, kk, :],
                            start=False,
                            stop=(k == NK - 1),
                        )
                    mn = stat_pool.tile([128, 1], F32, tag="mn", name="mn")
                    nc.vector.tensor_reduce(
                        mn[:, :], ps1[mi][:, :],
                        axis=mybir.AxisListType.X, op=mybir.AluOpType.min,
                    )
                    a_t = a_pool.tile([128, 512], BF16, tag="attn", name="attn")
                    s_t = stat_pool.tile([128, 1], F32, tag="s", name="s")
                    nc.scalar.activation(
                        a_t[:, :], ps1[mi][:, :],
                        mybir.ActivationFunctionType.Exp,
                        bias=mn[:, :], scale=-1.0, accum_out=s_t[:, :],
                    )
                    a_ts.append(a_t)
                    rs = stat_pool.tile([128, 1], F32, tag="rs", name="rs")
                    nc.vector.reciprocal(rs[:, :], s_t[:, :])
                    rg_t = stat_pool.tile([128, 1], F32, tag="rg", name="rg")
                    nc.vector.tensor_tensor(
                        rg_t[:, :], rs[:, :], gb[:, :], op=mybir.AluOpType.mult
                    )
                    rgs.append(rg_t)
                # attention^T via PE transpose-mode; PSUM slots from ps2 pool
                # (empty between batches), drained to SBUF by tiny DVE copies.
                at = [[None] * CB for _ in range(CB)]
                for mi in range(CB):
                    for dj in range(CB):
                        pst = ps2_pool.tile([128, 128], BF16, tag="ps2", name="atp")
                        nc.tensor.transpose(
                            pst[:, :],
                            a_ts[mi][:, dj * 128 : (dj + 1) * 128],
                            ident[:, :],
                        )
                        t_sb = at_pool.tile([128, 128], BF16, tag="att", name="att")
                        nc.vector.tensor_copy(t_sb[:, :], pst[:, :])
                        at[mi][dj] = t_sb
                return rgs, at

            def mm2_round(b, nt, rgs, at, stage):
                """One nt column of mm2 + fused epilogue; store every 2 rounds."""
                hh = nt // 4
                off = (nt % 4) * 512
                for mi in range(CB):
                    if nt % 2 == 0:
                        stage[mi] = stage_pool.tile(
                            [128, 1024], F32, tag="stage", name="stage"
                        )
                    ps2 = ps2_pool.tile([128, 512], F32, tag="ps2", name="ps2")
                    for dj in range(CB):
                        nc.tensor.matmul(
                            ps2[:, :],
                            at[mi][dj][:, :],
                            qbf[b][dj, hh][:, off : off + 512],
                            start=(dj == 0),
                            stop=(dj == CB - 1),
                        )
                    xsl = x_tiles[b][mi, nt // 2][
                        :, (nt % 2) * 512 : (nt % 2 + 1) * 512
                    ]
                    dst = stage[mi][:, (nt % 2) * 512 : (nt % 2 + 1) * 512]
                    nc.vector.scalar_tensor_tensor(
                        dst, ps2[:, :], rgs[mi][:, :], xsl,
                        op0=mybir.AluOpType.mult, op1=mybir.AluOpType.add,
                    )
                    if nt % 2 == 1:
                        nc.scalar.dma_start(
                            out[b, mi * 128 : (mi + 1) * 128,
                                (nt - 1) * 512 : (nt + 1) * 512],
                            stage[mi][:, :],
                        )

            # ---- main schedule ----
            prep_half(0, 0)
            prep_half(0, 1)
            for b in range(BPC):
                rgs, at = mm1_softmax(b)
                stage = {}
                for nt in range(NT):
                    mm2_round(b, nt, rgs, at, stage)
                    # interleave next batch's prep where its slots free up
                    if b + 1 < BPC:
                        if nt == 3:
                            prep_half(b + 1, 0)
                        elif nt == NT - 1:
                            prep_half(b + 1, 1)

    nc.finalize()
    return nc


def _get_program():
    global _PROGRAM
    if _PROGRAM is None:
        _PROGRAM = _build_program()
    return _PROGRAM


def _run(x, gamma, trace=False, tmpdir=None):
    """x: [B, C, H, W] fp32, gamma: [1] fp32 -> ([B, C, H, W] fp32, exec_time_ns)"""
    x = np.ascontiguousarray(np.asarray(x, dtype=np.float32)).reshape(B, C, N)
    gamma = np.ascontiguousarray(np.asarray(gamma, dtype=np.float32)).reshape(1)
    nc = _get_program()
    in_maps = [
        {"x": x[i * BPC : (i + 1) * BPC], "gamma": gamma} for i in range(N_CORES)
    ]
    res = run_bass_kernel_spmd(
        nc, in_maps, list(range(N_CORES)), trace=trace, tmpdir=tmpdir
    )
    full = np.concatenate([res.results[i]["out"] for i in range(N_CORES)], axis=0)
    return full.reshape(B, C, H, W), res.exec_time_ns


def kernel(**inputs):
    out, _ = _run(inputs["x"], inputs["gamma"])
    return out


if __name__ == "__main__":
    rng = np.random.default_rng(0)
    x = rng.standard_normal((B, C, H, W), dtype=np.float32)
    gamma = np.zeros((1,), dtype=np.float32)
    out, t = _run(x, gamma)
    print("exec_time_ns:", t)
    print("max |out - x| (gamma=0):", np.abs(out - x).max())


# revision 5
# speedup vs baseline: 1.5845x; 1.2345x over previous
"""CAM (channel attention) kernel for Trainium2, SPMD over 8 NeuronCores.

Problem: x [16, 512, 64, 64] fp32, gamma [1] fp32.
  q = x.reshape(B, C, N);  energy = q @ q^T          (C x C, contract over N=4096)
  attention = softmax(max(energy, -1, keepdims) - energy, -1)
  out = attention @ q;  result = gamma * out + x

Sharding: data-parallel over batch. 16 batches / 8 cores = 2 batches per core.
gamma replicated. Each core computes its own C x C attention per batch.

Math: energy is symmetric, and
  softmax(m[c] - energy[c, :]) = exp(mn[c] - energy[c, :]) / sum(...)
with mn[c] = min_d energy[c, d] (jax softmax's internal max-shift turns the
row-max of (m - e) into the row-min of e). All exp args <= 0 -> no overflow.

All transposes run on the TensorEngine in transpose-mode (DMA xbar-transposes
measured ~8us each here and throttle the global in-flight DMA window, which
starves the x loads). mm1 is software-pipelined per 128-wide n-chunk:
  PE:  [transpose chunk k (4x 128x128)] [matmuls chunk k-1 (4x N=512)] ...
with tiny DVE copies draining each transposed chunk from PSUM to SBUF.

Engine layout:
  SP ring : x loads (plain, back-to-back)
  ACT ring: out stores; ACT compute: exp with fused row-sum
  DVE     : fp32->bf16 casts, qT/AT psum->sbuf copies, row-min,
            epilogue (psum*rg)+x as one fused scalar_tensor_tensor
  PE      : qT transposes, mm1, AT transposes, mm2
"""

import sys

if "/opt/trn_rl_repo" not in sys.path:
    sys.path.insert(0, "/opt/trn_rl_repo")

import numpy as np

import concourse.bacc as bacc
import concourse.mybir as mybir
import concourse.tile as tile
from concourse.bass_utils import run_bass_kernel_spmd
from concourse.masks import make_identity

# Problem constants (hardcoded; kernel.py must be self-contained).
B, C, H, W = 16, 512, 64, 64
N = H * W                      # 4096
N_CORES = 8
BPC = B // N_CORES             # batches per core = 2
CB = C // 128                  # c-blocks = 4
NK = N // 128                  # contraction chunks for mm1 = 32
NT = N // 512                  # mm2 output tiles per c-block = 8

F32 = mybir.dt.float32
BF16 = mybir.dt.bfloat16

_PROGRAM = None


def _build_program():
    nc = bacc.Bacc("TRN2", target_bir_lowering=False, debug=True)
    x = nc.declare_dram_parameter("x", [BPC, C, N], F32, isOutput=False)
    gamma = nc.declare_dram_parameter("gamma", [1], F32, isOutput=False)
    out = nc.declare_dram_parameter("out", [BPC, C, N], F32, isOutput=True)

    with tile.TileContext(nc) as tc:
        with (
            tc.tile_pool(name="xf", bufs=20) as xf_pool,
            tc.tile_pool(name="qbf", bufs=12) as qbf_pool,
            tc.tile_pool(name="qts", bufs=6) as qts_pool,
            tc.tile_pool(name="attn", bufs=4) as a_pool,
            tc.tile_pool(name="att", bufs=20) as at_pool,
            tc.tile_pool(name="stat", bufs=24) as stat_pool,
            tc.tile_pool(name="stage", bufs=4) as stage_pool,
            tc.tile_pool(name="const", bufs=1) as const_pool,
            tc.tile_pool(name="ps1", bufs=4, space="PSUM") as ps1_pool,
            tc.tile_pool(name="ps2", bufs=4, space="PSUM") as ps2_pool,
        ):
            # constants: gamma broadcast + identity for PE transposes
            gsb = const_pool.tile([1, 1], F32, tag="gsb", name="gsb")
            nc.sync.dma_start(gsb[:, :], gamma[None, :])
            gb = const_pool.tile([128, 1], F32, tag="gb", name="gb")
            nc.gpsimd.partition_broadcast(gb[:, :], gsb[:, :])
            ident = const_pool.tile([128, 128], BF16, tag="ident", name="ident")
            make_identity(nc, ident[:, :])

            # per-batch state
            x_tiles = [{} for _ in range(BPC)]
            qbf = [{} for _ in range(BPC)]

            def prep_half(b, h):
                """Load x[b] half h (slab-major for early availability), cast."""
                for ci in range(CB):
                    qbf[b][ci, h] = qbf_pool.tile(
                        [128, 2048], BF16, tag="qbf", name="qbf"
                    )
                for s in range(2):
                    for ci in range(CB):
                        ns = h * 2 + s
                        xt = xf_pool.tile([128, 1024], F32, tag="xf", name="xf")
                        nc.sync.dma_start(
                            xt[:, :],
                            x[b, ci * 128 : (ci + 1) * 128,
                              ns * 1024 : (ns + 1) * 1024],
                        )
                        x_tiles[b][ci, ns] = xt
                        nc.vector.tensor_copy(
                            qbf[b][ci, h][:, s * 1024 : (s + 1) * 1024], xt[:, :]
                        )

            def transpose_chunk(b, k):
                """qt_k[p, ci*128+c'] = q[ci*128+c', k*128+p] via 4 PE transposes."""
                h, kk = divmod(k, NK // 2)
                qt_k = qts_pool.tile([128, 512], BF16, tag="qts", name="qts")
                for ci in range(CB):
                    pst = ps2_pool.tile([128, 128], BF16, tag="ps2", name="qtp")
                    nc.tensor.transpose(
                        pst[:, :],
                        qbf[b][ci, h][:, kk * 128 : (kk + 1) * 128],
                        ident[:, :],
                    )
                    nc.vector.tensor_copy(
                        qt_k[:, ci * 128 : (ci + 1) * 128], pst[:, :]
                    )
                return qt_k

            def mm1_softmax(b):
                """energy -> softmax stats -> attention^T tiles (PE transpose)."""
                ps1 = [
                    ps1_pool.tile([128, 512], F32, tag="ps1", name="ps1")
                    for _ in range(CB)
                ]
                # software pipeline: transpose chunk k+1 while matmuling chunk k
                qt_cur = transpose_chunk(b, 0)
                for k in range(NK):
                    qt_next = transpose_chunk(b, k + 1) if k + 1 < NK else None
                    for mi in range(CB):
                        nc.tensor.matmul(
                            ps1[mi][:, :],
                            qt_cur[:, mi * 128 : (mi + 1) * 128],
                            qt_cur[:, :],
                            start=(k == 0),
                            stop=(k == NK - 1),
                        )
                    qt_cur = qt_next
                rgs, a_ts = [], []
                for mi in range(CB):
                    mn = stat_pool.tile([128, 1], F32, tag="mn", name="mn")
                    nc.vector.tensor_reduce(
                        mn[:, :], ps1[mi][:, :],
                        axis=mybir.AxisListType.X, op=mybir.AluOpType.min,
                    )
                    a_t = a_pool.tile([128, 512], BF16, tag="attn", name="attn")
                    s_t = stat_pool.tile([128, 1], F32, tag="s", name="s")
                    nc.scalar.activation(
                        a_t[:, :], ps1[mi][:, :],
                        mybir.ActivationFunctionType.Exp,
                        bias=mn[:, :], scale=-1.0, accum_out=s_t[:, :],
                    )
                    a_ts.append(a_t)
                    rs = stat_pool.tile([128, 1], F32, tag="rs", name="rs")
                    nc.vector.reciprocal(rs[:, :], s_t[:, :])
                    rg_t = stat_pool.tile([128, 1], F32, tag="rg", name="rg")
                    nc.vector.tensor_tensor(
                        rg_t[:, :], rs[:, :], gb[:, :], op=mybir.AluOpType.mult
                    )
                    rgs.append(rg_t)
                # attention^T via PE transpose-mode
                at = [[None] * CB for _ in range(CB)]
                for mi in range(CB):
                    for dj in range(CB):
                        pst = ps2_pool.tile([128, 128], BF16, tag="ps2", name="atp")
                        nc.tensor.transpose(
                            pst[:, :],
                            a_ts[mi][:, dj * 128 : (dj + 1) * 128],
                            ident[:, :],
                        )
                        t_sb = at_pool.tile([128, 128], BF16, tag="att", name="att")
                        nc.vector.tensor_copy(t_sb[:, :], pst[:, :])
                        at[mi][dj] = t_sb
                return rgs, at

            def mm2_round(b, nt, rgs, at, stage):
                """One nt column of mm2 + fused epilogue; store every 2 rounds."""
                hh = nt // 4
                off = (nt % 4) * 512
                for mi in range(CB):
                    if nt % 2 == 0:
                        stage[mi] = stage_pool.tile(
                            [128, 1024], F32, tag="stage", name="stage"
                        )
                    ps2 = ps2_pool.tile([128, 512], F32, tag="ps2", name="ps2")
                    for dj in range(CB):
                        nc.tensor.matmul(
                            ps2[:, :],
                            at[mi][dj][:, :],
                            qbf[b][dj, hh][:, off : off + 512],
                            start=(dj == 0),
                            stop=(dj == CB - 1),
                        )
                    xsl = x_tiles[b][mi, nt // 2][
                        :, (nt % 2) * 512 : (nt % 2 + 1) * 512
                    ]
                    dst = stage[mi][:, (nt % 2) * 512 : (nt % 2 + 1) * 512]
                    nc.vector.scalar_tensor_tensor(
                        dst, ps2[:, :], rgs[mi][:, :], xsl,
                        op0=mybir.AluOpType.mult, op1=mybir.AluOpType.add,
                    )
                    if nt % 2 == 1:
                        nc.scalar.dma_start(
                            out[b, mi * 128 : (mi + 1) * 128,
                                (nt - 1) * 512 : (nt + 1) * 512],
                            stage[mi][:, :],
                        )

            # ---- main schedule ----
            prep_half(0, 0)
            prep_half(0, 1)
            for b in range(BPC):
                rgs, at = mm1_softmax(b)
                stage = {}
                for nt in range(NT):
                    mm2_round(b, nt, rgs, at, stage)
                    # interleave next batch's prep where its slots free up
                    if b + 1 < BPC:
                        if nt == 3:
                            prep_half(b + 1, 0)
                        elif nt == NT - 1:
                            prep_half(b + 1, 1)

    nc.finalize()
    return nc


def _get_program():
    global _PROGRAM
    if _PROGRAM is None:
        _PROGRAM = _build_program()
    return _PROGRAM


def _run(x, gamma, trace=False, tmpdir=None):
    """x: [B, C, H, W] fp32, gamma: [1] fp32 -> ([B, C, H, W] fp32, exec_time_ns)"""
    x = np.ascontiguousarray(np.asarray(x, dtype=np.float32)).reshape(B, C, N)
    gamma = np.ascontiguousarray(np.asarray(gamma, dtype=np.float32)).reshape(1)
    nc = _get_program()
    in_maps = [
        {"x": x[i * BPC : (i + 1) * BPC], "gamma": gamma} for i in range(N_CORES)
    ]
    res = run_bass_kernel_spmd(
        nc, in_maps, list(range(N_CORES)), trace=trace, tmpdir=tmpdir
    )
    full = np.concatenate([res.results[i]["out"] for i in range(N_CORES)], axis=0)
    return full.reshape(B, C, H, W), res.exec_time_ns


def kernel(**inputs):
    out, _ = _run(inputs["x"], inputs["gamma"])
    return out


if __name__ == "__main__":
    rng = np.random.default_rng(0)
    x = rng.standard_normal((B, C, H, W), dtype=np.float32)
    gamma = np.zeros((1,), dtype=np.float32)
    out, t = _run(x, gamma)
    print("exec_time_ns:", t)
    print("max |out - x| (gamma=0):", np.abs(out - x).max())
